# revision 14
# baseline (speedup 1.0000x reference)
"""MoE routing kernel for Trainium2 (8 NeuronCores, batch-parallel).

Problem: nn_MoE_47278999994656.
  x [8, 256, 80, 80] f32 + gate Linear(256->5) + 5 experts
  (residual conv1x1 on each 128-ch half, gated by a sigmoid transform),
  top-1 masked-softmax gate => weights are EXACTLY one-hot, so
  out[b] = expert_{argmax_e logits[b,e]}(x[b]).

Sharding: data-parallel over batch, core i computes batch item i.
Each core: computes the gate on-chip (PE psum accumulation), builds the
selected expert's weights by a mask-weighted sum over the 5 experts
(measure-zero tie risk accepted), then runs the expert on the SBUF-resident
x with fp32r/bf16 matmuls.
"""

import numpy as np

import concourse.bacc as bacc_mod
import concourse.bass as bass
import concourse.mybir as mybir
import concourse.tile as tile
from concourse.bass import ts
from concourse.bass_utils import run_bass_kernel_spmd

B, C, H, W = 8, 256, 80, 80
HW = H * W          # 6400
HALF = 128
QUARTER = 64
E = 5
NCORES = 8

CHUNK = 400         # free-dim tile for expert layers (>=256 keeps fp32r at 1 cyc/row)
NCH = HW // CHUNK   # 16
BLK = 4             # chunks per block (weights stay loaded per layer within a block)
NBLK = NCH // BLK   # 4
DMACH = 800         # input DMA chunk columns
NDMA = HW // DMACH  # 8

# U_all free-dim layout (per expert, partition dim = 128):
#   [0:128)    (I + Wrgb)^T        [c, o]
#   [128:256)  (I + Wtir)^T        [c, o]
#   [256:320)  Wt1^T               [o, m]   (m = 64)
#   [320:448)  Wt2 replicated      [m, :]   rows 64:128 zero
#   448        brgb                [o]
#   449        btir                [o]
#   450        bt1                 [m]      rows 64:128 zero
#   451        bt2                 broadcast to all 128 rows
UF = 452
U_RGB = 0
U_TIR = 128
U_WT1 = 256
U_WT2 = 320
U_BRGB = 448
U_BTIR = 449
U_BT1 = 450
U_BT2 = 451

F32 = mybir.dt.float32
F32R = mybir.dt.float32r
BF16 = mybir.dt.bfloat16


def build_nc() -> bass.Bass:
    nc = bacc_mod.Bacc()

    x_d = nc.dram_tensor("x", [C, HW], F32, kind="ExternalInput")
    u_d = nc.dram_tensor("u", [HALF, E, UF], BF16, kind="ExternalInput")
    bias_d = nc.dram_tensor("bias", [HALF, E, 4], F32, kind="ExternalInput")
    wg_d = nc.dram_tensor("wg", [HALF, 2, E], F32, kind="ExternalInput")
    bg_d = nc.dram_tensor("bg", [1, E], F32, kind="ExternalInput")
    out_d = nc.dram_tensor("out", [HALF, HW], F32, kind="ExternalOutput")

    with tile.TileContext(nc) as tc:
        with (
            tc.tile_pool(name="big", bufs=1) as big,
            tc.tile_pool(name="const", bufs=1) as const,
            tc.tile_pool(name="small", bufs=1) as small,
            tc.tile_pool(name="hpool", bufs=4) as hpool,
            tc.tile_pool(name="ppool", bufs=4) as ppool,
            tc.tile_pool(name="gps", bufs=1, space="PSUM") as gps,
            tc.tile_pool(name="dps_p", bufs=2, space="PSUM") as dps_p,
            tc.tile_pool(name="hps_p", bufs=2, space="PSUM") as hps_p,
            tc.tile_pool(name="aps_p", bufs=2, space="PSUM") as aps_p,
        ):
            # ---- persistent SBUF ----
            xs = big.tile([HALF, 2, HW], F32)        # 51.2 KB/part
            xb = big.tile([HALF, 2, HW], BF16)       # 12.8 KB/part
            dsb = big.tile([HALF, 2, HW], BF16)      # 25.6 KB/part
            osb = big.tile([HALF, HW], BF16)         # 12.8 KB/part
            ssb_t = big.tile([HALF, 2, HW], BF16)    # 25.6 KB/part
            u_all = const.tile([HALF, E, UF], BF16)  # 4.5 KB/part
            bias_all = const.tile([HALF, E, 4], F32)
            wg = const.tile([HALF, 2, E], F32)
            bg = const.tile([1, E], F32)

            nc.sync.dma_start(out=u_all[:], in_=u_d[:])
            nc.sync.dma_start(out=bias_all[:], in_=bias_d[:])
            nc.sync.dma_start(out=wg[:], in_=wg_d[:])
            nc.sync.dma_start(out=bg[:], in_=bg_d[:])

            # ---- phase 1: stream x in; DVE partial row-sums for the gate ----
            partials = small.tile([HALF, 2, NDMA], F32)
            for h in range(2):
                for j in range(NDMA):
                    sl = ts(j, DMACH)
                    nc.sync.dma_start(
                        out=xs[:, h, sl], in_=x_d[h * HALF : (h + 1) * HALF, sl]
                    )
                    nc.vector.reduce_sum(
                        partials[:, h, j : j + 1], xs[:, h, sl],
                        axis=mybir.AxisListType.X,
                    )
                    nc.scalar.copy(xb[:, h, sl], xs[:, h, sl])

            pooled = small.tile([HALF, 2], F32)
            nc.vector.reduce_sum(pooled, partials, axis=mybir.AxisListType.X)
            wgs = small.tile([HALF, 2, E], F32)
            nc.vector.tensor_copy(wgs, wg)

            # logits (scaled by HW): exact fp32 matmul, K=128 per half
            lps = gps.tile([E, 1], F32, tag="g")
            for h in range(2):
                nc.tensor.matmul(
                    lps, lhsT=wgs[:, h, :], rhs=pooled[:, h : h + 1],
                    start=(h == 0), stop=(h == 1),
                )
            l51 = small.tile([E, 1], F32)
            nc.vector.tensor_copy(l51, lps)

            t32a = small.tile([32, 32], F32)
            t32b = small.tile([32, 32], F32)
            nc.vector.memset(t32a, 0.0)
            nc.vector.tensor_copy(t32a[0:E, 0:1], l51)
            nc.vector.transpose(t32b, t32a)
            lrow = t32b[0:1, 0:E]
            nc.vector.tensor_add(lrow, lrow, bg)   # bg pre-scaled by HW on host
            lmax = small.tile([1, 1], F32)
            nc.vector.reduce_max(lmax, lrow, axis=mybir.AxisListType.X)
            mrow = small.tile([1, E], F32)
            nc.vector.tensor_scalar(
                out=mrow, in0=lrow, scalar1=lmax, scalar2=None,
                op0=mybir.AluOpType.is_equal,
            )
            ones1 = small.tile([1, HALF], F32)
            nc.vector.memset(ones1, 1.0)
            mps = gps.tile([HALF, E], F32, tag="g")
            nc.tensor.matmul(mps, lhsT=ones1, rhs=mrow)
            mbc = small.tile([HALF, E], F32)
            nc.vector.tensor_copy(mbc, mps)

            # ---- select expert weights: U_sel = sum_e mask[e] * U_all[e] ----
            usel = small.tile([HALF, UF], BF16)
            utmp = small.tile([HALF, UF], BF16)
            nc.vector.tensor_scalar_mul(usel, u_all[:, 0, :], mbc[:, 0:1])
            for e in range(1, E):
                nc.vector.tensor_scalar_mul(utmp, u_all[:, e, :], mbc[:, e : e + 1])
                nc.vector.tensor_add(usel, usel, utmp)
            bsel = small.tile([HALF, 4], F32)
            btmp = small.tile([HALF, 4], F32)
            nc.vector.tensor_scalar_mul(bsel, bias_all[:, 0, :], mbc[:, 0:1])
            for e in range(1, E):
                nc.vector.tensor_scalar_mul(btmp, bias_all[:, e, :], mbc[:, e : e + 1])
                nc.vector.tensor_add(bsel, bsel, btmp)

            # single-wait legalization: PE matmuls carry at most one sync
            # wait, so pre-observe cross-engine ticks on cheap instructions.
            bscr = small.tile([HALF, 4], F32)
            nc.scalar.copy(bscr, bsel)                   # ACT observes DVE(bsel)
            obs = gps.tile([1, 1], F32, tag="g")
            nc.tensor.matmul(                            # PE observes ACT(xb)
                obs, lhsT=xb[:, 1, HW - 1 : HW], rhs=xb[:, 1, HW - 1 : HW]
            )

            # ---- phase 2: selected expert, blocked over HW ----
            for blk in range(NBLK):
                js = [blk * BLK + i for i in range(BLK)]
                # D layer: D = (I+W)@x -> bf16 psum; evac +bias to SBUF
                for s in range(2):
                    for j in js:
                        sl = ts(j, CHUNK)
                        dps = dps_p.tile([HALF, CHUNK], F32, tag="dps")
                        nc.tensor.matmul(
                            dps,
                            lhsT=usel[:, s * HALF : (s + 1) * HALF],
                            rhs=xb[:, s, sl],
                        )
                        nc.vector.tensor_scalar_add(
                            dsb[:, s, sl], dps,
                            bsel[:, s : s + 1],
                        )
                # H layer: H = relu(Wt1@D + bt1)
                hsbs = {}
                for s in range(2):
                    for j in js:
                        sl = ts(j, CHUNK)
                        hps = hps_p.tile([QUARTER, CHUNK], F32, tag="hps")
                        nc.tensor.matmul(
                            hps,
                            lhsT=usel[:, U_WT1 : U_WT1 + QUARTER],
                            rhs=dsb[:, s, sl],
                        )
                        hsb = hpool.tile([QUARTER, CHUNK], BF16, tag="hsb")
                        nc.scalar.activation(
                            out=hsb, in_=hps,
                            func=mybir.ActivationFunctionType.Relu,
                            bias=bsel[0:QUARTER, 2:3],
                        )
                        hsbs[(s, j)] = hsb
                # A layer (broadcast to 128 partitions) + sigmoid
                for s in range(2):
                    for j in js:
                        aps = aps_p.tile([HALF, CHUNK], F32, tag="aps")
                        nc.tensor.matmul(
                            aps,
                            lhsT=usel[0:QUARTER, U_WT2 : U_WT2 + HALF],
                            rhs=hsbs[(s, j)],
                        )
                        nc.scalar.activation(
                            out=ssb_t[:, s, ts(j, CHUNK)], in_=aps,
                            func=mybir.ActivationFunctionType.Sigmoid,
                            bias=bsel[:, 3:4],
                        )
                # combine: out = S_r*D_r + S_t*D_t
                for j in js:
                    sl = ts(j, CHUNK)
                    prt = ppool.tile([HALF, CHUNK], BF16, tag="prt")
                    nc.vector.tensor_mul(prt, dsb[:, 0, sl], ssb_t[:, 0, sl])
                    nc.vector.tensor_mul(osb[:, sl], dsb[:, 1, sl], ssb_t[:, 1, sl])
                    nc.vector.tensor_add(osb[:, sl], osb[:, sl], prt)
                # store block (SWDGE casts bf16 -> f32)
                bsl = ts(blk, BLK * CHUNK)
                nc.gpsimd.dma_start(out=out_d[:, bsl], in_=osb[:, bsl])


    nc.compile()
    return nc


def _pack_inputs(x, Wg, bg, Wrgb, brgb, Wtir, btir, Wt1, bt1, Wt2, bt2):
    eye = np.eye(HALF, dtype=np.float32)
    u = np.zeros((E, HALF, UF), dtype=np.float32)
    for e in range(E):
        u[e, :, U_RGB : U_RGB + HALF] = Wrgb[e].T + eye
        u[e, :, U_TIR : U_TIR + HALF] = Wtir[e].T + eye
        u[e, :, U_WT1 : U_WT1 + QUARTER] = Wt1[e].T
        u[e, 0:QUARTER, U_WT2 : U_WT2 + HALF] = np.repeat(
            Wt2[e, 0][:, None], HALF, axis=1
        )
        u[e, :, U_BRGB] = brgb[e]
        u[e, :, U_BTIR] = btir[e]
        u[e, 0:QUARTER, U_BT1] = bt1[e]
        u[e, :, U_BT2] = bt2[e, 0]
    import ml_dtypes
    u = np.ascontiguousarray(u.transpose(1, 0, 2)).astype(ml_dtypes.bfloat16)

    wgt = Wg.T.astype(np.float32)                   # [256, 5]
    wg_p = np.ascontiguousarray(
        np.stack([wgt[:HALF], wgt[HALF:]], axis=1)  # [128, 2, 5]
    )
    bg_p = np.ascontiguousarray((bg * float(HW))[None, :].astype(np.float32))

    bias = np.zeros((E, HALF, 4), dtype=np.float32)
    for e in range(E):
        bias[e, :, 0] = brgb[e]
        bias[e, :, 1] = btir[e]
        bias[e, 0:QUARTER, 2] = bt1[e]
        bias[e, :, 3] = bt2[e, 0]
    bias = np.ascontiguousarray(bias.transpose(1, 0, 2))
    common = {"u": u, "bias": bias, "wg": wg_p, "bg": bg_p}
    in_maps = []
    for b in range(B):
        m = dict(common)
        m["x"] = np.ascontiguousarray(x[b].reshape(C, HW).astype(np.float32))
        in_maps.append(m)
    return in_maps


_NC_CACHE = {}


def _get_nc():
    if "nc" not in _NC_CACHE:
        _NC_CACHE["nc"] = build_nc()
    return _NC_CACHE["nc"]


def kernel(x, Wg, bg, Wrgb, brgb, Wtir, btir, Wt1, bt1, Wt2, bt2, **run_kw):
    nc = _get_nc()
    in_maps = _pack_inputs(
        np.asarray(x), np.asarray(Wg), np.asarray(bg), np.asarray(Wrgb),
        np.asarray(brgb), np.asarray(Wtir), np.asarray(btir),
        np.asarray(Wt1), np.asarray(bt1), np.asarray(Wt2), np.asarray(bt2),
    )
    res = run_bass_kernel_spmd(nc, in_maps, core_ids=list(range(NCORES)), **run_kw)
    out = np.stack([r["out"] for r in res.results], axis=0)  # [8, 128, 6400]
    if run_kw:
        kernel.last_results = res
    return out.reshape(B, HALF, H, W).astype(np.float32)
